# revision 4
# baseline (speedup 1.0000x reference)
"""Trainium2 Bass kernel for the Potts-discriminator energy model.

Math (reference):
    Xf = X.reshape(B, D)                     # B=64, D=L*N=2688
    j_sum[b]  = sum_ij Xf[b,i] J[i,j] Xf[b,j]
    h_sum[b]  = Xf[b,:] @ H_w + H_b
    energy    = j_sum + h_sum
    out       = sigmoid(energy)
    reg_j     = sum(J**2); reg_h = sum(H_w**2)

Sharding: J is column-sharded across 8 cores (336 cols each).  Core c computes
    G_c = Xf @ J[:, cols_c]                  # [B, 336] via 21 K=128 matmuls
    partial_c[b] = sum_j G_c[b,j] * Xf[b, cols_c][j]
plus sum-of-squares of its J shard.  H_w is appended as a 337th column so
Xf @ H_w falls out of the same matmul.  Host sums the 8 per-sample partials
(the "all-reduce"), adds the bias, and applies the sigmoid on 64 scalars.

Schedule notes (from NTFF traces):
  - DMA issue costs ~650ns each on the issuing engine; inputs are spread
    across both HWDGE queues (sync + scalar) to parallelize issue.
  - fp32 matmul lowers to LOW/HIGH dual-pass; the PE must be HAM-warm
    (2.4 GHz) or it costs 2x.  Dummy matmuls on constant tiles warm it
    up while input DMAs are in flight.
  - tensor_tensor_reduce traps this runtime; dot uses mul + reduce.
"""

import os

import numpy as np

B = 64
L = 128
NS = 21
D = L * NS            # 2688
NCORES = 8
CPC = D // NCORES     # 336 columns of J per core
KT = D // 128         # 21 contraction tiles of 128
NAUG = CPC + 1        # 337: J columns + H_w column
CHUNKS = [5, 5, 5, 5, 1]   # K-tiles per DMA chunk
NCH = len(CHUNKS)
DUMMY_MM = 6          # PE warm-up matmuls
DUMMY_N = 256

_STATE = {}           # holds the compiled Bass module across calls

# Results of the last device run (for test harnesses to inspect profiling).
LAST_RESULTS = None


def _build_module():
    import concourse.bacc as bacc
    import concourse.tile as tile
    from concourse import mybir

    f32 = mybir.dt.float32
    nc = bacc.Bacc("TRN2", target_bir_lowering=False, debug=False,
                   num_devices=NCORES)

    xfth_d = nc.dram_tensor("xfth", (128, KT, B + 1), f32,
                            kind="ExternalInput").ap()
    jsb_d = nc.dram_tensor("jsb", (128, KT, NAUG), f32,
                           kind="ExternalInput").ap()
    xfc_d = nc.dram_tensor("xfc", (B, CPC), f32, kind="ExternalInput").ap()
    out_d = nc.dram_tensor("out", (B, 3), f32, kind="ExternalOutput").ap()

    with tile.TileContext(nc) as tc:
        with (
            tc.tile_pool(name="persist", bufs=1) as persist,
            tc.tile_pool(name="psum", bufs=1, space="PSUM") as psum,
            tc.tile_pool(name="scratch", bufs=2) as scratch,
        ):
            stage = persist.tile([B, 3], f32, tag="stage")
            nc.gpsimd.memset(stage[:], 0.0)
            ones = persist.tile([128, 1], f32, tag="ones")
            nc.gpsimd.memset(ones[:], 1.0)
            dummy_rhs = persist.tile([128, DUMMY_N], f32, tag="dummy_rhs")
            nc.gpsimd.memset(dummy_rhs[:], 0.0)

            # Input DMAs, spread across the two HWDGE queues.
            xfth = persist.tile([128, KT, B + 1], f32, tag="xfth")
            nc.sync.dma_start(xfth[:], xfth_d[:])
            chunks = []
            k0 = 0
            for c, ch in enumerate(CHUNKS):
                jc = persist.tile([128, ch, NAUG], f32, tag=f"jchunk{c}")
                eng = nc.scalar if c % 2 == 0 else nc.sync
                eng.dma_start(jc[:], jsb_d[:, k0:k0 + ch, :])
                chunks.append((jc, k0, ch))
                k0 += ch
            xfc = persist.tile([B, CPC], f32, tag="xfc")
            nc.scalar.dma_start(xfc[:], xfc_d[:])

            # PE warm-up: keep TensorE busy while input DMAs fly so the
            # HAM clock-gate opens before the real matmuls.
            warm_ps = psum.tile([1, DUMMY_N], f32, tag="warm")
            for _ in range(DUMMY_MM):
                nc.tensor.matmul(warm_ps[:], ones[:], dummy_rhs[:],
                                 start=True, stop=True)

            g_ps = psum.tile([B, NAUG], f32, tag="g")
            sq_acc = persist.tile([128, NCH], f32, tag="sq_acc")
            for c, (jc, k0, ch) in enumerate(chunks):
                for i in range(ch):
                    n = k0 + i
                    nc.tensor.matmul(
                        g_ps[:],
                        xfth[:, n, 0:B],        # lhsT [K=128, M=64]
                        jc[:, i, :],            # rhs  [K=128, N=337]
                        start=(n == 0),
                        stop=(n == KT - 1),
                    )
                sq_out = scratch.tile([128, ch, NAUG], f32, tag=f"sq_out{ch}")
                nc.scalar.activation(
                    sq_out[:], jc[:],
                    mybir.ActivationFunctionType.Square,
                    accum_out=sq_acc[:, c:c + 1],
                )

            # partial_j[b] = sum_j G[b, :336] * Xf_cols[b, :]
            dot_out = scratch.tile([B, CPC], f32, tag="dot_out")
            nc.vector.tensor_mul(dot_out[:], g_ps[:, 0:CPC], xfc[:])
            nc.vector.tensor_reduce(
                out=stage[:, 0:1], in_=dot_out[:],
                axis=mybir.AxisListType.X, op=mybir.AluOpType.add,
            )
            # h_pre[b] = (Xf @ H_w)[b]  (column 336 of the augmented matmul)
            nc.vector.tensor_copy(stage[:, 1:2], g_ps[:, CPC:CPC + 1])

            # per-partition sums: [:,0] = sumsq of J shard (incl. H col),
            #                     [:,1] = sumsq of H_w
            regs2 = persist.tile([128, 2], f32, tag="regs2")
            nc.vector.tensor_reduce(
                out=regs2[:, 0:1], in_=sq_acc[:],
                axis=mybir.AxisListType.X, op=mybir.AluOpType.add,
            )
            hsq_out = scratch.tile([128, KT], f32, tag="hsq_out")
            nc.scalar.activation(
                hsq_out[:], xfth[:, :, B:B + 1],
                mybir.ActivationFunctionType.Square,
                accum_out=regs2[:, 1:2],
            )
            # cross-partition reduce: [2,1] = regs2.T @ ones
            reg_ps = psum.tile([2, 1], f32, tag="regps")
            nc.tensor.matmul(reg_ps[:], regs2[:], ones[:], start=True,
                             stop=True)
            nc.vector.tensor_copy(stage[0:2, 2:3], reg_ps[:])

            nc.sync.dma_start(out_d[:], stage[:])

    nc.compile()
    return nc


def _prepare_in_maps(X, J_w, H_w):
    Xf = np.ascontiguousarray(X.reshape(B, D), dtype=np.float32)
    # xfth[p, n, m] = Xf[m, n*128 + p];  xfth[p, n, 64] = H_w[n*128 + p]
    xft = Xf.T.reshape(KT, 128, B).transpose(1, 0, 2)
    hw2 = H_w.reshape(KT, 128).T
    xfth = np.ascontiguousarray(
        np.concatenate([xft, hw2[:, :, None]], axis=2), dtype=np.float32)
    in_maps = []
    for c in range(NCORES):
        cols = slice(c * CPC, (c + 1) * CPC)
        jaug = np.concatenate(
            [J_w[:, cols], H_w[:, None]], axis=1)          # [D, 337]
        jsb = np.ascontiguousarray(
            jaug.reshape(KT, 128, NAUG).transpose(1, 0, 2),
            dtype=np.float32)                              # [128, KT, 337]
        xfc = np.ascontiguousarray(Xf[:, cols])
        in_maps.append({"xfth": xfth, "jsb": jsb, "xfc": xfc})
    return in_maps


def kernel(X, J_w, H_w, H_b):
    global LAST_RESULTS
    from concourse.bass_utils import run_bass_kernel_spmd

    if "nc" not in _STATE:
        _STATE["nc"] = _build_module()
    nc = _STATE["nc"]

    in_maps = _prepare_in_maps(
        np.asarray(X, dtype=np.float32),
        np.asarray(J_w, dtype=np.float32),
        np.asarray(H_w, dtype=np.float32),
    )
    trace = bool(os.environ.get("KERNEL_TRACE"))
    res = run_bass_kernel_spmd(nc, in_maps, core_ids=list(range(NCORES)),
                               trace=trace)
    LAST_RESULTS = res

    outs = np.stack([r["out"] for r in res.results])       # [8, 64, 3]
    partial_j = outs[:, :, 0].sum(axis=0)                  # [64]
    h_pre = outs[0, :, 1]                                  # [64]
    energy = (partial_j + h_pre + np.float32(np.asarray(H_b).reshape(-1)[0])
              ).astype(np.float32)
    sig = (1.0 / (1.0 + np.exp(-energy.astype(np.float64)))).astype(np.float32)
    reg_h = outs[0, 1, 2]
    reg_j = outs[:, 0, 2].sum() - NCORES * reg_h
    return (sig, energy,
            np.asarray(reg_j, dtype=np.float32),
            np.asarray(reg_h, dtype=np.float32))


# revision 6
# speedup vs baseline: 1.0778x; 1.0778x over previous
"""Trainium2 Bass kernel for the Potts-discriminator energy model.

Math (reference):
    Xf = X.reshape(B, D)                     # B=64, D=L*N=2688
    j_sum[b]  = sum_ij Xf[b,i] J[i,j] Xf[b,j]
    h_sum[b]  = Xf[b,:] @ H_w + H_b
    energy    = j_sum + h_sum
    out       = sigmoid(energy)
    reg_j     = sum(J**2); reg_h = sum(H_w**2)

Sharding: J is column-sharded across 8 cores (336 cols each).  Core c computes
    G_c = Xf @ J[:, cols_c]                  # [B, 336] via 21 K=128 matmuls
    partial_c[b] = sum_j G_c[b,j] * Xf[b, cols_c][j]
plus sum-of-squares of its J shard.  H_w is appended as a 337th column so
Xf @ H_w falls out of the same matmul.  Host sums the 8 per-sample partials
(the "all-reduce"), adds the bias, and applies the sigmoid on 64 scalars.

Schedule notes (from NTFF traces):
  - DMA issue costs ~650ns each on the issuing engine; inputs are spread
    across both HWDGE queues (sync + scalar) to parallelize issue.
  - fp32 matmul lowers to LOW/HIGH dual-pass; the PE must be HAM-warm
    (2.4 GHz) or it costs 2x.  Dummy matmuls on constant tiles warm it
    up while input DMAs are in flight.
  - tensor_tensor_reduce traps this runtime; dot uses mul + reduce.
"""

import os

import numpy as np

B = 64
L = 128
NS = 21
D = L * NS            # 2688
NCORES = 8
CPC = D // NCORES     # 336 columns of J per core
KT = D // 128         # 21 contraction tiles of 128
NAUG = CPC + 1        # 337: J columns + H_w column
CHUNKS = [2, 3, 4, 4, 4, 4]   # K-tiles per DMA chunk (ramp: latency -> bw)
NCH = len(CHUNKS)
DUMMY_MM = 8          # PE warm-up matmuls
DUMMY_N = 256

_STATE = {}           # holds the compiled Bass module across calls

# Results of the last device run (for test harnesses to inspect profiling).
LAST_RESULTS = None


def _build_module():
    import concourse.bacc as bacc
    import concourse.tile as tile
    from concourse import mybir

    f32 = mybir.dt.float32
    nc = bacc.Bacc("TRN2", target_bir_lowering=False, debug=False,
                   num_devices=NCORES)

    xfth_d = nc.dram_tensor("xfth", (128, KT, B + 1), f32,
                            kind="ExternalInput").ap()
    jsb_d = nc.dram_tensor("jsb", (128, KT, NAUG), f32,
                           kind="ExternalInput").ap()
    xfc_d = nc.dram_tensor("xfc", (B, CPC), f32, kind="ExternalInput").ap()
    out_d = nc.dram_tensor("out", (B, 3), f32, kind="ExternalOutput").ap()

    with tile.TileContext(nc) as tc:
        with (
            tc.tile_pool(name="persist", bufs=1) as persist,
            tc.tile_pool(name="psum", bufs=1, space="PSUM") as psum,
            tc.tile_pool(name="scratch", bufs=2) as scratch,
        ):
            stage = persist.tile([B, 3], f32, tag="stage")
            nc.gpsimd.memset(stage[:], 0.0)
            ones = persist.tile([128, 1], f32, tag="ones")
            nc.gpsimd.memset(ones[:], 1.0)
            dummy_rhs = persist.tile([128, DUMMY_N], f32, tag="dummy_rhs")
            nc.gpsimd.memset(dummy_rhs[:], 0.0)

            # Input DMAs all on one HWDGE ring (sync): per-ring FIFO makes
            # completions progressive at full HBM bandwidth, so each J
            # chunk lands just ahead of the matmuls that consume it.
            # (Two rings split bandwidth round-robin and delay the first
            # chunk's completion past the PE warm-up window.)
            xfth = persist.tile([128, KT, B + 1], f32, tag="xfth")
            nc.sync.dma_start(xfth[:], xfth_d[:])
            chunks = []
            k0 = 0
            for c, ch in enumerate(CHUNKS):
                jc = persist.tile([128, ch, NAUG], f32, tag=f"jchunk{c}")
                nc.sync.dma_start(jc[:], jsb_d[:, k0:k0 + ch, :])
                chunks.append((jc, k0, ch))
                k0 += ch
            xfc = persist.tile([B, CPC], f32, tag="xfc")
            nc.sync.dma_start(xfc[:], xfc_d[:])

            # PE warm-up: keep TensorE busy while input DMAs fly so the
            # HAM clock-gate opens before the real matmuls.
            warm_ps = psum.tile([1, DUMMY_N], f32, tag="warm")
            for _ in range(DUMMY_MM):
                nc.tensor.matmul(warm_ps[:], ones[:], dummy_rhs[:],
                                 start=True, stop=True)

            g_ps = psum.tile([B, NAUG], f32, tag="g")
            sq_acc = persist.tile([128, NCH], f32, tag="sq_acc")
            for c, (jc, k0, ch) in enumerate(chunks):
                for i in range(ch):
                    n = k0 + i
                    nc.tensor.matmul(
                        g_ps[:],
                        xfth[:, n, 0:B],        # lhsT [K=128, M=64]
                        jc[:, i, :],            # rhs  [K=128, N=337]
                        start=(n == 0),
                        stop=(n == KT - 1),
                    )
                sq_out = scratch.tile([128, ch, NAUG], f32, tag=f"sq_out{ch}")
                nc.scalar.activation(
                    sq_out[:], jc[:],
                    mybir.ActivationFunctionType.Square,
                    accum_out=sq_acc[:, c:c + 1],
                )

            # partial_j[b] = sum_j G[b, :336] * Xf_cols[b, :]
            dot_out = scratch.tile([B, CPC], f32, tag="dot_out")
            nc.vector.tensor_mul(dot_out[:], g_ps[:, 0:CPC], xfc[:])
            nc.vector.tensor_reduce(
                out=stage[:, 0:1], in_=dot_out[:],
                axis=mybir.AxisListType.X, op=mybir.AluOpType.add,
            )
            # h_pre[b] = (Xf @ H_w)[b]  (column 336 of the augmented matmul)
            nc.vector.tensor_copy(stage[:, 1:2], g_ps[:, CPC:CPC + 1])

            # per-partition sums: [:,0] = sumsq of J shard (incl. H col),
            #                     [:,1] = sumsq of H_w
            regs2 = persist.tile([128, 2], f32, tag="regs2")
            nc.vector.tensor_reduce(
                out=regs2[:, 0:1], in_=sq_acc[:],
                axis=mybir.AxisListType.X, op=mybir.AluOpType.add,
            )
            hsq_out = scratch.tile([128, KT], f32, tag="hsq_out")
            nc.scalar.activation(
                hsq_out[:], xfth[:, :, B:B + 1],
                mybir.ActivationFunctionType.Square,
                accum_out=regs2[:, 1:2],
            )
            # cross-partition reduce: [2,1] = regs2.T @ ones
            reg_ps = psum.tile([2, 1], f32, tag="regps")
            nc.tensor.matmul(reg_ps[:], regs2[:], ones[:], start=True,
                             stop=True)
            nc.vector.tensor_copy(stage[0:2, 2:3], reg_ps[:])

            nc.sync.dma_start(out_d[:], stage[:])

    nc.compile()
    return nc


def _prepare_in_maps(X, J_w, H_w):
    Xf = np.ascontiguousarray(X.reshape(B, D), dtype=np.float32)
    # xfth[p, n, m] = Xf[m, n*128 + p];  xfth[p, n, 64] = H_w[n*128 + p]
    xft = Xf.T.reshape(KT, 128, B).transpose(1, 0, 2)
    hw2 = H_w.reshape(KT, 128).T
    xfth = np.ascontiguousarray(
        np.concatenate([xft, hw2[:, :, None]], axis=2), dtype=np.float32)
    in_maps = []
    for c in range(NCORES):
        cols = slice(c * CPC, (c + 1) * CPC)
        jaug = np.concatenate(
            [J_w[:, cols], H_w[:, None]], axis=1)          # [D, 337]
        jsb = np.ascontiguousarray(
            jaug.reshape(KT, 128, NAUG).transpose(1, 0, 2),
            dtype=np.float32)                              # [128, KT, 337]
        xfc = np.ascontiguousarray(Xf[:, cols])
        in_maps.append({"xfth": xfth, "jsb": jsb, "xfc": xfc})
    return in_maps


def kernel(X, J_w, H_w, H_b):
    global LAST_RESULTS
    from concourse.bass_utils import run_bass_kernel_spmd

    if "nc" not in _STATE:
        _STATE["nc"] = _build_module()
    nc = _STATE["nc"]

    in_maps = _prepare_in_maps(
        np.asarray(X, dtype=np.float32),
        np.asarray(J_w, dtype=np.float32),
        np.asarray(H_w, dtype=np.float32),
    )
    trace = bool(os.environ.get("KERNEL_TRACE"))
    res = run_bass_kernel_spmd(nc, in_maps, core_ids=list(range(NCORES)),
                               trace=trace)
    LAST_RESULTS = res

    outs = np.stack([r["out"] for r in res.results])       # [8, 64, 3]
    partial_j = outs[:, :, 0].sum(axis=0)                  # [64]
    h_pre = outs[0, :, 1]                                  # [64]
    energy = (partial_j + h_pre + np.float32(np.asarray(H_b).reshape(-1)[0])
              ).astype(np.float32)
    sig = (1.0 / (1.0 + np.exp(-energy.astype(np.float64)))).astype(np.float32)
    reg_h = outs[0, 1, 2]
    reg_j = outs[:, 0, 2].sum() - NCORES * reg_h
    return (sig, energy,
            np.asarray(reg_j, dtype=np.float32),
            np.asarray(reg_h, dtype=np.float32))
